# revision 20
# baseline (speedup 1.0000x reference)
"""Trainium2 Bass kernel for nn_CrossAttention_8435315769719.

CrossAttention block: LN(x), LN(context), 12-head query / single shared
KV head cross-attention, output projection, plus a parallel SwiGLU FF on
the normed x.

Sharding: the 4096 query tokens (4 batches x 1024) are split into 8
row-slices of 512 tokens; core c handles batch c//2, token rows
(c%2)*512..+512, with that batch's full context. KV projection is
recomputed per core (cheap); no collectives needed.

All matmul operands are bf16 (same PE rate as float32r at N>=256 but
half the DMA/SBUF footprint); accumulation stays fp32 in PSUM. LN
statistics are computed in fp32. The SwiGLU FF chunks are interleaved
into the attention phase to keep the PE dense (and at max p-state)
while the scalar engine works through the softmax Exps.
"""

import functools

import numpy as np
import ml_dtypes

import concourse.bass as bass
import concourse.tile as tile
from concourse import mybir
from concourse.bass_utils import run_bass_kernel_spmd

# ---------------------------------------------------------------- sizes
DIM = 768
HEADS = 12
DH = 64
FFI = 3072  # FF inner (per u/gate half)
B = 4
N = 1024
J = 2048  # context length
EPS = 1e-5
NCORES = 8
TPC = 512  # query tokens per core

KC = DIM // 128  # 6 contraction chunks
TT = TPC // 128  # 4 token tiles per core
JT = J // 128  # 16 context tiles
FH = FFI // 128  # 24 ff tiles per half

F32 = mybir.dt.float32
F32R = mybir.dt.float32r
BF16 = mybir.dt.bfloat16
I16 = mybir.dt.int16

SUB = mybir.AluOpType.subtract
MULT = mybir.AluOpType.mult
ADD = mybir.AluOpType.add
AF = mybir.ActivationFunctionType


# ------------------------------------------------- tile drain workaround
def _install_drain_patch():
    """walrus in this container rejects >1 sync-wait on the Tile tail
    Drain ("Too many sync wait commands"). Split the final global-clock
    waits onto individual SP nops instead."""
    import concourse.tile as _t

    if getattr(_t.TileContext, "_drain_patched", False):
        return

    def _patched(self, tick_clock, wait_clock):
        nc = self.nc
        drain_inst = nc.sync.drain()
        wait_clock.add_sem_waits(
            drain_inst.ins, _t.ScopedClock({None: tick_clock.global_clock})
        )
        si = drain_inst.ins.sync_info
        waits = list(si.on_wait) if si is not None else []
        if si is not None and len(waits) > 1:
            si.on_wait = []
            for w in waits:
                n = nc.sync.nop(nofuse=True, hint="drain_split")
                n.ins.sync_info = mybir.SyncInfo(on_wait=[w], on_update=[])
        nc.all_engine_barrier()
        assert self.sems is not None
        popped = nc._tile_sem_poison_stack.pop()
        assert popped is self._sem_poison
        nc.clear_and_free_semaphores(list(self.sems.allocated().values()))
        nc.all_engine_barrier()

    _t.TileContext._drain_and_barrier = _patched
    _t.TileContext._drain_patched = True


# ------------------------------------------------------------ LN helper
def _ln_stats(nc, pool, xt, eps_t):
    """bn_stats/bn_aggr mean+var over the 768-wide free dim (3x256
    subgroups), returns (mu, var) [128,1] APs."""
    xg = xt.rearrange("p (s d) -> p s d", d=256)
    nsub = xg.shape[1]
    stats = pool.tile([128, nsub, nc.vector.BN_STATS_DIM], F32, tag="bnst")
    for s in range(nsub):
        nc.vector.bn_stats(out=stats[:, s, :], in_=xg[:, s, :])
    mv = pool.tile([128, nc.vector.BN_AGGR_DIM], F32, tag="mv")
    nc.vector.bn_aggr(out=mv, in_=stats)
    return mv[:, 0:1], mv[:, 1:2]


# -------------------------------------------------------- program build
def _emit(nc):
    x_d = nc.dram_tensor("x", [TPC, DIM], BF16, kind="ExternalInput").ap()
    ctx_d = nc.dram_tensor("ctx", [J, DIM], BF16, kind="ExternalInput").ap()
    wq_d = nc.dram_tensor("wq", [DIM, DIM], BF16, kind="ExternalInput").ap()
    wkv_d = nc.dram_tensor("wkv", [DIM, 2 * DH], BF16, kind="ExternalInput").ap()
    wo_d = nc.dram_tensor("wo", [DIM, DIM], BF16, kind="ExternalInput").ap()
    wff1_d = nc.dram_tensor("wff1", [DIM, 2 * FFI], BF16, kind="ExternalInput").ap()
    wff2_d = nc.dram_tensor("wff2", [FFI, DIM], BF16, kind="ExternalInput").ap()
    ident_d = nc.dram_tensor("ident", [128, 128], BF16, kind="ExternalInput").ap()
    y_d = nc.dram_tensor("y", [TPC, DIM], F32, kind="ExternalOutput").ap()

    wq_r = wq_d.rearrange("(k p) n -> p k n", p=128)
    wkv_r = wkv_d.rearrange("(k p) n -> p k n", p=128)
    wo_r = wo_d.rearrange("(k p) n -> p k n", p=128)
    wff1_r = wff1_d.rearrange("(k p) n -> p k n", p=128)

    with tile.TileContext(nc) as tc:
        _build_tile(tc, nc, x_d, ctx_d, wq_r, wkv_r, wo_r, wff1_r, wff2_d, y_d,
                    ident_d)
    _split_excess_waits(nc)
    return nc


def _split_excess_waits(nc, max_waits=1):
    """walrus in this container rejects instructions carrying more than
    ~2 sync waits ("Too many sync wait commands"). Move the excess onto
    same-engine NOPs inserted immediately before the instruction."""
    for fn in nc.m.functions:
        for bb in fn.blocks:
            rebuilt = []
            changed = False
            for inst in bb.instructions:
                si = inst.sync_info
                waits = list(si.on_wait) if si is not None else []
                if len(waits) > max_waits:
                    changed = True
                    si.on_wait = waits[-max_waits:]
                    excess = waits[:-max_waits]
                    for i0 in range(0, len(excess), max_waits):
                        nop = mybir.InstNoOp(
                            name=nc.get_next_instruction_name(),
                            sync_info=mybir.SyncInfo(
                                on_wait=excess[i0 : i0 + max_waits], on_update=[]
                            ),
                            bass_nofuse=True,
                            engine=inst.engine,
                        )
                        nc.register_instruction(nop, overwrite=True)
                        rebuilt.append(nop)
                rebuilt.append(inst)
            if changed:
                bb.instructions = rebuilt


def _build_tile(tc, nc, x_d, ctx_d, wq_r, wkv_r, wo_r, wff1_r, wff2_d, y_d,
                ident_d):
    from contextlib import ExitStack

    ctx = ExitStack()
    with ctx:
        constp = ctx.enter_context(tc.tile_pool(name="const", bufs=1))
        pers = ctx.enter_context(tc.tile_pool(name="pers", bufs=1))

        ident = constp.tile([128, 128], BF16)
        nc.sync.dma_start(out=ident, in_=ident_d)
        eps_t = constp.tile([128, 1], F32)
        nc.vector.memset(eps_t, EPS)
        # eye2 @ sums2 broadcasts head0's denom row to partitions 0:64
        # and head1's to 64:128, in a single matmul
        eye2 = constp.tile([128, 128], BF16)
        nc.gpsimd.memset(eye2, 0.0)
        nc.gpsimd.memset(eye2[0:1, 0:DH], 1.0)
        nc.gpsimd.memset(eye2[64:65, DH:128], 1.0)

        xnT = pers.tile([128, KC, TPC], BF16)  # LN(x)^T     6KB/part
        outT = pers.tile([128, KC, TPC], BF16)  # attn out^T  6KB/part

        def ln_group(dram, t0, iop, stp):
            # LN for 4 row-tiles with ONE batched reciprocal (InstReciprocal
            # has a flat ~1.3us cost regardless of free size)
            parts = []
            std4 = stp.tile([128, 4], F32, tag="std4")
            for j in range(4):
                t = t0 + j
                xt = iop.tile([128, DIM], BF16, tag="ln_in", name=f"ln{t}")
                nc.sync.dma_start(out=xt, in_=dram[t * 128 : (t + 1) * 128, :])
                mu, var = _ln_stats(nc, stp, xt, eps_t)
                nc.scalar.activation(
                    out=std4[:, j : j + 1], in_=var, func=AF.Sqrt,
                    bias=eps_t, scale=1.0,
                )
                parts.append((xt, mu))
            rs4 = stp.tile([128, 4], F32, tag="rs4")
            nc.vector.reciprocal(out=rs4, in_=std4)
            outs = []
            for j, (xt, mu) in enumerate(parts):
                nc.vector.tensor_scalar(
                    out=xt, in0=xt, scalar1=mu, scalar2=rs4[:, j : j + 1],
                    op0=SUB, op1=MULT,
                )
                outs.append(xt)
            return outs

        wop = ctx.enter_context(tc.tile_pool(name="wo", bufs=1))
        # SwiGLU hidden and Wff2 outlive the attention scope (used in G)
        htp = ctx.enter_context(tc.tile_pool(name="ht", bufs=1))
        hT = htp.tile([128, FH, TPC], BF16)  # swiglu hidden^T 24KB/part
        w2p = ctx.enter_context(tc.tile_pool(name="wff2", bufs=1))
        w2_sb = w2p.tile([128, FH, DIM], BF16)  # 36KB/part
        wff2_r = wff2_d.rearrange("(f p) n -> p f n", p=128)

        # attention working set, freed after phase F
        with tc.tile_pool(name="attn_data", bufs=1) as adp:
            qT = adp.tile([128, KC, TPC], BF16)  # q^T heads   6KB/part
            vT = adp.tile([64, J], BF16)  # v^T (dim-major)
            # k^T zero-padded to K=128: lo = [k; 0], hi = [0; k].
            # (K=64 / M=65 matmuls measured ~2.5x slower than K=M=128.)
            kTd_lo = adp.tile([128, J], BF16)
            kTd_hi = adp.tile([128, J], BF16)
            vaug = adp.tile([128, JT, 2 * DH], BF16)  # v | ones | zero pad
            # pad memsets on gpsimd: keeps the vector queue free for LN
            nc.gpsimd.memset(kTd_lo[64:128, :], 0.0)
            nc.gpsimd.memset(kTd_hi[0:64, :], 0.0)
            nc.gpsimd.memset(vaug, 0.0)
            nc.gpsimd.memset(vaug[:, :, DH : DH + 1], 1.0)

            with (
                tc.tile_pool(name="io", bufs=6) as iop,
                tc.tile_pool(name="stats", bufs=8) as stp,
                tc.tile_pool(name="tpp", bufs=4, space="PSUM") as tpp,
                tc.tile_pool(name="mmp", bufs=2, space="PSUM") as mmp,
                tc.tile_pool(name="wq", bufs=1) as wqp,
                tc.tile_pool(name="cnT", bufs=1) as cnp,
            ):
                def transpose_768(xt, dst, t):
                    # 6 transposes into one PSUM bank, one strided copy out
                    pt = tpp.tile([128, DIM], BF16, tag="tp")
                    for k in range(KC):
                        nc.tensor.matmul(
                            pt[:, k * 128 : (k + 1) * 128],
                            lhsT=xt[:, k * 128 : (k + 1) * 128],
                            rhs=ident,
                            is_transpose=True,
                        )
                    nc.scalar.copy(
                        out=dst[:, :, t * 128 : (t + 1) * 128],
                        in_=pt.rearrange("p (k c) -> p k c", c=128),
                    )

                # ---------- phase A: LN(x) -> xnT ------------------------
                for t, xt in enumerate(ln_group(x_d, 0, iop, stp)):
                    transpose_768(xt, xnT, t)

                # weights: wq right after x tiles, before the ctx stream
                wq_sb = wqp.tile([128, KC, DIM], BF16)  # 9KB/part
                nc.sync.dma_start(out=wq_sb, in_=wq_r)
                wkv_sb = constp.tile([128, KC, 2 * DH], BF16)
                nc.sync.dma_start(out=wkv_sb, in_=wkv_r)

                # ---------- phase C: qT = Wq^T @ xn^T --------------------
                for i in range(KC):
                    ps = mmp.tile([128, 512], F32, tag="mm")
                    for k in range(KC):
                        nc.tensor.matmul(
                            ps,
                            lhsT=wq_sb[:, k, i * 128 : (i + 1) * 128],
                            rhs=xnT[:, k, :],
                            start=(k == 0),
                            stop=(k == KC - 1),
                        )
                    nc.vector.tensor_copy(out=qT[:, i, :], in_=ps)

                # ---------- phase B+D+E: LN(ctx) -> cnT -> kv chunks -----
                cnT = cnp.tile([128, KC, J], BF16)  # 24KB/part
                for n in range(JT // 4):
                    for j, ct in enumerate(ln_group(ctx_d, 4 * n, iop, stp)):
                        transpose_768(ct, cnT, 4 * n + j)
                    if True:
                        # kv for the 512 ctx rows whose cnT is now complete
                        sl = slice(n * 512, (n + 1) * 512)
                        ps = mmp.tile([128, 512], F32, tag="mm")
                        for k in range(KC):
                            nc.tensor.matmul(
                                ps,
                                lhsT=wkv_sb[:, k, :],
                                rhs=cnT[:, k, sl],
                                start=(k == 0),
                                stop=(k == KC - 1),
                            )
                        nc.vector.tensor_copy(out=kTd_lo[0:64, sl], in_=ps[0:64, :])
                        nc.scalar.copy(out=kTd_hi[64:128, sl], in_=ps[0:64, :])
                        nc.scalar.copy(out=vT[:, sl], in_=ps[64:128, :])
                        # v to token-major (vaug) as soon as each kv chunk lands
                        pt = tpp.tile([128, 4 * DH], BF16, tag="tp")
                        for jj in range(4):
                            nc.tensor.matmul(
                                pt[:, jj * DH : (jj + 1) * DH],
                                lhsT=vT[:, (4 * n + jj) * 128 : (4 * n + jj + 1) * 128],
                                rhs=ident[0:64, 0:64],
                                is_transpose=True,
                            )
                        nc.scalar.copy(
                            out=vaug[:, 4 * n : 4 * n + 4, 0:DH],
                            in_=pt.rearrange("p (j c) -> p j c", c=DH),
                        )

            # prefetch Wo during attention (pool opened before attn_data)
            wo_sb = wop.tile([128, KC, DIM], BF16)  # 9KB/part
            nc.sync.dma_start(out=wo_sb, in_=wo_r)

            # ---------- phase F+H: attention with FF interleaved ---------
            with (
                tc.tile_pool(name="spsum", bufs=4, space="PSUM") as spsum,
                tc.tile_pool(name="opsum", bufs=1, space="PSUM") as opsum,
                tc.tile_pool(name="hps", bufs=1, space="PSUM") as hps,
                tc.tile_pool(name="attn", bufs=5) as apool,
                tc.tile_pool(name="rb", bufs=2) as rbp,
                tc.tile_pool(name="wff1", bufs=4) as w1p,
                tc.tile_pool(name="sil", bufs=3) as silp,
            ):
                def ff_chunk(f):
                    # one 128-col chunk of the SwiGLU hidden; fills the PE
                    # while the scalar engine chews the softmax Exps
                    nc.sync.dma_start(out=w2_sb[:, f, :], in_=wff2_r[:, f, :])
                    wg = w1p.tile([128, KC, 128], BF16, tag="wg")
                    nc.sync.dma_start(
                        out=wg,
                        in_=wff1_r[:, :, FFI + f * 128 : FFI + (f + 1) * 128],
                    )
                    pg = hps.tile([128, 512], F32, tag="pg")
                    for k in range(KC):
                        nc.tensor.matmul(
                            pg,
                            lhsT=wg[:, k, :],
                            rhs=xnT[:, k, :],
                            start=(k == 0),
                            stop=(k == KC - 1),
                        )
                    sil = silp.tile([128, 512], BF16, tag="sil")
                    nc.scalar.activation(out=sil, in_=pg, func=AF.Silu)
                    wu = w1p.tile([128, KC, 128], BF16, tag="wu")
                    nc.sync.dma_start(
                        out=wu, in_=wff1_r[:, :, f * 128 : (f + 1) * 128]
                    )
                    pu = hps.tile([128, 512], F32, tag="pu")
                    for k in range(KC):
                        nc.tensor.matmul(
                            pu,
                            lhsT=wu[:, k, :],
                            rhs=xnT[:, k, :],
                            start=(k == 0),
                            stop=(k == KC - 1),
                        )
                    nc.vector.tensor_mul(out=hT[:, f, :], in0=pu, in1=sil)

                # Schraudolph fast-exp constants for bf16: bitcast of
                # int16(x*184.665 + 16249) approximates e^x to ~3% — the
                # random per-weight error washes out in the softmax average.
                S_A = 184.6650292
                S_B = 16249.0

                for hp in range(HEADS // 2):
                    po = [
                        opsum.tile([128, 512], F32, tag=f"o{i}", name=f"po{hp}_{i}")
                        for i in range(2)
                    ]
                    # blocks of 4 j-tiles: 8 consecutive sim matmuls, then
                    # per-head runs of 4 consecutive av matmuls into one
                    # PSUM bank (consecutive same-group matmuls run at the
                    # chain rate; per-instruction group/bank alternation
                    # measured ~100ns/MM slower)
                    JB = 4
                    for jb in range(0, JT, JB):
                        ats = {}
                        for jt in range(jb, jb + JB):
                            for i in range(2):
                                kTd = kTd_lo if i == 0 else kTd_hi
                                ps = spsum.tile([128, 512], F32, tag="s")
                                nc.tensor.matmul(
                                    ps,
                                    lhsT=kTd[:, jt * 128 : (jt + 1) * 128],
                                    rhs=qT[:, hp, :],
                                    start=True,
                                    stop=True,
                                )
                                if i == 0:
                                    # scalar-engine exact exp
                                    at = apool.tile([128, 512], BF16, tag="a0")
                                    nc.scalar.activation(
                                        out=at, in_=ps, func=AF.Exp
                                    )
                                else:
                                    # vector-engine fast exp (Schraudolph)
                                    ai = apool.tile([128, 512], I16, tag="a1")
                                    nc.vector.tensor_scalar(
                                        out=ai, in0=ps, scalar1=S_A,
                                        scalar2=S_B, op0=MULT, op1=ADD,
                                    )
                                    at = ai.bitcast(BF16)
                                ats[(jt, i)] = at
                        for i in range(2):
                            for jt in range(jb, jb + JB):
                                nc.tensor.matmul(
                                    po[i],
                                    lhsT=vaug[:, jt, :],
                                    rhs=ats[(jt, i)],
                                    start=(jt == 0),
                                    stop=(jt == JT - 1),
                                )
                    # tail: scalar copies pack both heads' attn-out and
                    # denom rows (freeing the po banks fast), then ONE
                    # broadcast matmul + reciprocal + multiply normalizes
                    # both heads at once
                    pos = rbp.tile([128, 512], F32, tag="pos")
                    sums2 = rbp.tile([128, 512], BF16, tag="sums2")
                    for i in range(2):
                        nc.scalar.copy(
                            out=pos[i * 64 : (i + 1) * 64, :], in_=po[i][0:DH, :]
                        )
                        nc.scalar.copy(
                            out=sums2[i * 64 : i * 64 + 1, :],
                            in_=po[i][DH : DH + 1, :],
                        )
                    pb = spsum.tile([128, 512], F32, tag="s")
                    nc.tensor.matmul(
                        pb,
                        lhsT=eye2,
                        rhs=sums2,
                        start=True,
                        stop=True,
                    )
                    rb = rbp.tile([128, 512], F32, tag="rbb")
                    nc.vector.reciprocal(out=rb, in_=pb)
                    nc.vector.tensor_mul(out=outT[:, hp, :], in0=pos, in1=rb)
                    # 4 FF chunks per head-pair: silus stay grouped so the
                    # scalar engine reloads the act table only 2x per hp
                    for f in range(4 * hp, 4 * hp + 4):
                        ff_chunk(f)

        # ---------- phase G: out = outT.T@Wo + hT.T@Wff2 -----------------
        NCH = ((0, 512), (512, 256))  # 768 = 512 + 256, bank-aligned slices
        with (
            tc.tile_pool(name="gps", bufs=2, space="PSUM") as gps,
            tc.tile_pool(name="yout", bufs=2) as yp,
        ):
            for t in range(TT):
                pg = gps.tile([128, DIM], F32, tag="g")
                for n0, nw in NCH:
                    # Wff2 part first: hT is ready before the attention
                    # tail, so these chains don't wait on outT
                    for f in range(FH):
                        nc.tensor.matmul(
                            pg[:, n0 : n0 + nw],
                            lhsT=hT[:, f, t * 128 : (t + 1) * 128],
                            rhs=w2_sb[:, f, n0 : n0 + nw],
                            start=(f == 0),
                            stop=False,
                        )
                    for k in range(KC):
                        nc.tensor.matmul(
                            pg[:, n0 : n0 + nw],
                            lhsT=outT[:, k, t * 128 : (t + 1) * 128],
                            rhs=wo_sb[:, k, n0 : n0 + nw],
                            start=False,
                            stop=(k == KC - 1),
                        )
                ysb = yp.tile([128, DIM], F32, tag="y")
                nc.vector.tensor_copy(out=ysb, in_=pg)
                nc.sync.dma_start(out=y_d[t * 128 : (t + 1) * 128, :], in_=ysb)


@functools.lru_cache(maxsize=1)
def _build():
    _install_drain_patch()
    nc = bass.Bass("TRN2", target_bir_lowering=False, debug=False, num_devices=NCORES)
    return _emit(nc)


# ------------------------------------------------------ ntff hook shim
def _ensure_ntff_hook():
    """This image's `antenv` lacks `axon_hooks`; synthesize it so
    run_bass_kernel_spmd(trace=True) can capture NTFF profiles via the
    libaxon_pjrt C ABI (same recipe as trn_boot._ntff_profile_via_ctypes)."""
    import contextlib
    import ctypes
    import os
    import sys
    import types

    try:
        from antenv.axon_hooks import get_axon_ntff_profile_hook  # noqa: F401

        return
    except ImportError:
        pass
    import antenv

    mod = types.ModuleType("antenv.axon_hooks")
    holder = {"hook": None}
    mod.set_axon_ntff_profile_hook = lambda h: holder.__setitem__("hook", h)
    mod.get_axon_ntff_profile_hook = lambda: holder["hook"]
    sys.modules["antenv.axon_hooks"] = mod
    antenv.axon_hooks = mod

    so_path = "/opt/axon/libaxon_pjrt.so"
    if not os.path.exists(so_path):
        return
    lib = ctypes.CDLL(so_path)
    if not hasattr(lib, "axon_start_nrt_profile"):
        return
    lib.axon_start_nrt_profile.argtypes = [
        ctypes.POINTER(ctypes.c_int64),
        ctypes.c_size_t,
    ]
    lib.axon_start_nrt_profile.restype = ctypes.c_int64
    lib.axon_stop_nrt_profile.argtypes = [ctypes.c_char_p]
    lib.axon_stop_nrt_profile.restype = ctypes.c_int64

    @contextlib.contextmanager
    def _hook(output_dir, device_ids):
        import jax

        jax.devices()
        if device_ids:
            ids = (ctypes.c_int64 * len(device_ids))(*device_ids)
            rc = lib.axon_start_nrt_profile(ids, len(device_ids))
        else:
            rc = lib.axon_start_nrt_profile(None, 0)
        if rc != 0:
            raise RuntimeError(f"axon_start_nrt_profile rc={rc}")
        try:
            yield
        finally:
            n = lib.axon_stop_nrt_profile(str(output_dir).encode())
            print(f"ntff profile: {n} file(s) written to {output_dir}")

    mod.set_axon_ntff_profile_hook(_hook)


# ---------------------------------------------------------------- entry
TRACE = False  # test harnesses can flip this to capture an NTFF profile
LAST = None
BF = ml_dtypes.bfloat16
IDENT = np.eye(128, dtype=BF)


def kernel(**inputs):
    x = np.asarray(inputs["x"], dtype=np.float32)
    context = np.asarray(inputs["context"], dtype=np.float32)
    gx = np.asarray(inputs["gamma_x"], dtype=np.float32)
    gc = np.asarray(inputs["gamma_c"], dtype=np.float32)
    scale = DH**-0.5
    # fold LN gammas and the attention scale into the first-layer weights
    wq = ((gx[:, None] * np.asarray(inputs["Wq"])) * scale).astype(BF)
    wkv = (gc[:, None] * np.asarray(inputs["Wkv"])).astype(BF)
    wff1 = (gx[:, None] * np.asarray(inputs["Wff1"])).astype(BF)
    wo = np.asarray(inputs["Wo"], dtype=np.float32).astype(BF)
    wff2 = np.asarray(inputs["Wff2"], dtype=np.float32).astype(BF)
    xb = np.ascontiguousarray(x.astype(BF))
    cb = np.ascontiguousarray(context.astype(BF))

    in_maps = []
    for c in range(NCORES):
        b, t0 = c // 2, (c % 2) * TPC
        in_maps.append(
            {
                "x": np.ascontiguousarray(xb[b, t0 : t0 + TPC]),
                "ctx": cb[b],
                "wq": wq,
                "wkv": wkv,
                "wo": wo,
                "wff1": wff1,
                "wff2": wff2,
                "ident": IDENT,
            }
        )

    nc = _build()
    if TRACE:
        _ensure_ntff_hook()
    res = run_bass_kernel_spmd(nc, in_maps, list(range(NCORES)), trace=TRACE)
    global LAST
    LAST = res
    out = np.empty((B, N, DIM), np.float32)
    for c in range(NCORES):
        b, t0 = c // 2, (c % 2) * TPC
        out[b, t0 : t0 + TPC] = res.results[c]["y"]
    return out


# revision 21
# speedup vs baseline: 1.1880x; 1.1880x over previous
"""Trainium2 Bass kernel for nn_CrossAttention_8435315769719.

CrossAttention block: LN(x), LN(context), 12-head query / single shared
KV head cross-attention, output projection, plus a parallel SwiGLU FF on
the normed x.

Sharding: the 4096 query tokens (4 batches x 1024) are split into 8
row-slices of 512 tokens; core c handles batch c//2, token rows
(c%2)*512..+512, with that batch's full context. KV projection is
recomputed per core (cheap); no collectives needed.

All matmul operands are bf16 (same PE rate as float32r at N>=256 but
half the DMA/SBUF footprint); accumulation stays fp32 in PSUM. LN
statistics are computed in fp32. The SwiGLU FF chunks are interleaved
into the attention phase to keep the PE dense (and at max p-state)
while the scalar engine works through the softmax Exps.
"""

import functools

import numpy as np
import ml_dtypes

import concourse.bass as bass
import concourse.tile as tile
from concourse import mybir
from concourse.bass_utils import run_bass_kernel_spmd

# ---------------------------------------------------------------- sizes
DIM = 768
HEADS = 12
DH = 64
FFI = 3072  # FF inner (per u/gate half)
B = 4
N = 1024
J = 2048  # context length
EPS = 1e-5
NCORES = 8
TPC = 512  # query tokens per core

KC = DIM // 128  # 6 contraction chunks
TT = TPC // 128  # 4 token tiles per core
JT = J // 128  # 16 context tiles
FH = FFI // 128  # 24 ff tiles per half

F32 = mybir.dt.float32
F32R = mybir.dt.float32r
BF16 = mybir.dt.bfloat16
I16 = mybir.dt.int16

SUB = mybir.AluOpType.subtract
MULT = mybir.AluOpType.mult
ADD = mybir.AluOpType.add
AF = mybir.ActivationFunctionType


# ------------------------------------------------- tile drain workaround
def _install_drain_patch():
    """walrus in this container rejects >1 sync-wait on the Tile tail
    Drain ("Too many sync wait commands"). Split the final global-clock
    waits onto individual SP nops instead."""
    import concourse.tile as _t

    if getattr(_t.TileContext, "_drain_patched", False):
        return

    def _patched(self, tick_clock, wait_clock):
        nc = self.nc
        drain_inst = nc.sync.drain()
        wait_clock.add_sem_waits(
            drain_inst.ins, _t.ScopedClock({None: tick_clock.global_clock})
        )
        si = drain_inst.ins.sync_info
        waits = list(si.on_wait) if si is not None else []
        if si is not None and len(waits) > 1:
            si.on_wait = []
            for w in waits:
                n = nc.sync.nop(nofuse=True, hint="drain_split")
                n.ins.sync_info = mybir.SyncInfo(on_wait=[w], on_update=[])
        nc.all_engine_barrier()
        assert self.sems is not None
        popped = nc._tile_sem_poison_stack.pop()
        assert popped is self._sem_poison
        nc.clear_and_free_semaphores(list(self.sems.allocated().values()))
        nc.all_engine_barrier()

    _t.TileContext._drain_and_barrier = _patched
    _t.TileContext._drain_patched = True


# ------------------------------------------------------------ LN helper
def _ln_stats(nc, pool, xt, eps_t):
    """bn_stats/bn_aggr mean+var over the 768-wide free dim (3x256
    subgroups), returns (mu, var) [128,1] APs."""
    xg = xt.rearrange("p (s d) -> p s d", d=256)
    nsub = xg.shape[1]
    stats = pool.tile([128, nsub, nc.vector.BN_STATS_DIM], F32, tag="bnst")
    for s in range(nsub):
        nc.vector.bn_stats(out=stats[:, s, :], in_=xg[:, s, :])
    mv = pool.tile([128, nc.vector.BN_AGGR_DIM], F32, tag="mv")
    nc.vector.bn_aggr(out=mv, in_=stats)
    return mv[:, 0:1], mv[:, 1:2]


# -------------------------------------------------------- program build
def _emit(nc):
    x_d = nc.dram_tensor("x", [TPC, DIM], BF16, kind="ExternalInput").ap()
    ctx_d = nc.dram_tensor("ctx", [J, DIM], BF16, kind="ExternalInput").ap()
    wq_d = nc.dram_tensor("wq", [DIM, DIM], BF16, kind="ExternalInput").ap()
    wkv_d = nc.dram_tensor("wkv", [DIM, 2 * DH], BF16, kind="ExternalInput").ap()
    wo_d = nc.dram_tensor("wo", [DIM, DIM], BF16, kind="ExternalInput").ap()
    wff1_d = nc.dram_tensor("wff1", [DIM, 2 * FFI], BF16, kind="ExternalInput").ap()
    wff2_d = nc.dram_tensor("wff2", [FFI, DIM], BF16, kind="ExternalInput").ap()
    ident_d = nc.dram_tensor("ident", [128, 128], BF16, kind="ExternalInput").ap()
    y_d = nc.dram_tensor("y", [TPC, DIM], F32, kind="ExternalOutput").ap()

    wq_r = wq_d.rearrange("(k p) n -> p k n", p=128)
    wkv_r = wkv_d.rearrange("(k p) n -> p k n", p=128)
    wo_r = wo_d.rearrange("(k p) n -> p k n", p=128)
    wff1_r = wff1_d.rearrange("(k p) n -> p k n", p=128)

    with tile.TileContext(nc) as tc:
        _build_tile(tc, nc, x_d, ctx_d, wq_r, wkv_r, wo_r, wff1_r, wff2_d, y_d,
                    ident_d)
    _split_excess_waits(nc)
    return nc


def _split_excess_waits(nc, max_waits=1):
    """walrus in this container rejects instructions carrying more than
    ~2 sync waits ("Too many sync wait commands"). Move the excess onto
    same-engine NOPs inserted immediately before the instruction."""
    for fn in nc.m.functions:
        for bb in fn.blocks:
            rebuilt = []
            changed = False
            for inst in bb.instructions:
                si = inst.sync_info
                waits = list(si.on_wait) if si is not None else []
                if len(waits) > max_waits:
                    changed = True
                    si.on_wait = waits[-max_waits:]
                    excess = waits[:-max_waits]
                    for i0 in range(0, len(excess), max_waits):
                        nop = mybir.InstNoOp(
                            name=nc.get_next_instruction_name(),
                            sync_info=mybir.SyncInfo(
                                on_wait=excess[i0 : i0 + max_waits], on_update=[]
                            ),
                            bass_nofuse=True,
                            engine=inst.engine,
                        )
                        nc.register_instruction(nop, overwrite=True)
                        rebuilt.append(nop)
                rebuilt.append(inst)
            if changed:
                bb.instructions = rebuilt


def _build_tile(tc, nc, x_d, ctx_d, wq_r, wkv_r, wo_r, wff1_r, wff2_d, y_d,
                ident_d):
    from contextlib import ExitStack

    ctx = ExitStack()
    with ctx:
        constp = ctx.enter_context(tc.tile_pool(name="const", bufs=1))
        pers = ctx.enter_context(tc.tile_pool(name="pers", bufs=1))

        ident = constp.tile([128, 128], BF16)
        nc.sync.dma_start(out=ident, in_=ident_d)
        eps_t = constp.tile([128, 1], F32)
        nc.vector.memset(eps_t, EPS)
        # eye2 @ sums2 broadcasts head0's denom row to partitions 0:64
        # and head1's to 64:128, in a single matmul
        eye2 = constp.tile([128, 128], BF16)
        nc.gpsimd.memset(eye2, 0.0)
        nc.gpsimd.memset(eye2[0:1, 0:DH], 1.0)
        nc.gpsimd.memset(eye2[64:65, DH:128], 1.0)

        xnT = pers.tile([128, KC, TPC], BF16)  # LN(x)^T     6KB/part
        outT = pers.tile([128, KC, TPC], BF16)  # attn out^T  6KB/part

        def ln_group(dram, t0, iop, stp):
            # LN for 4 row-tiles with ONE batched reciprocal (InstReciprocal
            # has a flat ~1.3us cost regardless of free size)
            parts = []
            std4 = stp.tile([128, 4], F32, tag="std4")
            for j in range(4):
                t = t0 + j
                xt = iop.tile([128, DIM], BF16, tag="ln_in", name=f"ln{t}")
                nc.sync.dma_start(out=xt, in_=dram[t * 128 : (t + 1) * 128, :])
                mu, var = _ln_stats(nc, stp, xt, eps_t)
                nc.scalar.activation(
                    out=std4[:, j : j + 1], in_=var, func=AF.Sqrt,
                    bias=eps_t, scale=1.0,
                )
                parts.append((xt, mu))
            rs4 = stp.tile([128, 4], F32, tag="rs4")
            nc.vector.reciprocal(out=rs4, in_=std4)
            outs = []
            for j, (xt, mu) in enumerate(parts):
                nc.vector.tensor_scalar(
                    out=xt, in0=xt, scalar1=mu, scalar2=rs4[:, j : j + 1],
                    op0=SUB, op1=MULT,
                )
                outs.append(xt)
            return outs

        wop = ctx.enter_context(tc.tile_pool(name="wo", bufs=1))
        # SwiGLU hidden and Wff2 outlive the attention scope (used in G)
        htp = ctx.enter_context(tc.tile_pool(name="ht", bufs=1))
        hT = htp.tile([128, FH, TPC], BF16)  # swiglu hidden^T 24KB/part
        w2p = ctx.enter_context(tc.tile_pool(name="wff2", bufs=1))
        w2_sb = w2p.tile([128, FH, DIM], BF16)  # 36KB/part
        wff2_r = wff2_d.rearrange("(f p) n -> p f n", p=128)

        # attention working set, freed after phase F
        with tc.tile_pool(name="attn_data", bufs=1) as adp:
            qT = adp.tile([128, KC, TPC], BF16)  # q^T heads   6KB/part
            vT = adp.tile([64, J], BF16)  # v^T (dim-major)
            # k^T zero-padded to K=128: lo = [k; 0], hi = [0; k].
            # (K=64 / M=65 matmuls measured ~2.5x slower than K=M=128.)
            kTd_lo = adp.tile([128, J], BF16)
            kTd_hi = adp.tile([128, J], BF16)
            vaug = adp.tile([128, JT, 2 * DH], BF16)  # v | ones | zero pad
            # pad memsets on gpsimd: keeps the vector queue free for LN
            nc.gpsimd.memset(kTd_lo[64:128, :], 0.0)
            nc.gpsimd.memset(kTd_hi[0:64, :], 0.0)
            nc.gpsimd.memset(vaug, 0.0)
            nc.gpsimd.memset(vaug[:, :, DH : DH + 1], 1.0)

            with (
                tc.tile_pool(name="io", bufs=6) as iop,
                tc.tile_pool(name="stats", bufs=8) as stp,
                tc.tile_pool(name="tpp", bufs=4, space="PSUM") as tpp,
                tc.tile_pool(name="mmp", bufs=2, space="PSUM") as mmp,
                tc.tile_pool(name="wq", bufs=1) as wqp,
                tc.tile_pool(name="cnT", bufs=1) as cnp,
            ):
                def transpose_768(xt, dst, t):
                    # 6 transposes into one PSUM bank, one strided copy out
                    pt = tpp.tile([128, DIM], BF16, tag="tp")
                    for k in range(KC):
                        nc.tensor.matmul(
                            pt[:, k * 128 : (k + 1) * 128],
                            lhsT=xt[:, k * 128 : (k + 1) * 128],
                            rhs=ident,
                            is_transpose=True,
                        )
                    cp = nc.scalar.copy if t % 2 else nc.vector.tensor_copy
                    cp(
                        out=dst[:, :, t * 128 : (t + 1) * 128],
                        in_=pt.rearrange("p (k c) -> p k c", c=128),
                    )

                # ---------- phase A: LN(x) -> xnT ------------------------
                for t, xt in enumerate(ln_group(x_d, 0, iop, stp)):
                    transpose_768(xt, xnT, t)

                # weights: wq right after x tiles, before the ctx stream
                wq_sb = wqp.tile([128, KC, DIM], BF16)  # 9KB/part
                nc.sync.dma_start(out=wq_sb, in_=wq_r)
                wkv_sb = constp.tile([128, KC, 2 * DH], BF16)
                nc.sync.dma_start(out=wkv_sb, in_=wkv_r)

                # ---------- phase C: qT = Wq^T @ xn^T --------------------
                for i in range(KC):
                    ps = mmp.tile([128, 512], F32, tag="mm")
                    for k in range(KC):
                        nc.tensor.matmul(
                            ps,
                            lhsT=wq_sb[:, k, i * 128 : (i + 1) * 128],
                            rhs=xnT[:, k, :],
                            start=(k == 0),
                            stop=(k == KC - 1),
                        )
                    nc.vector.tensor_copy(out=qT[:, i, :], in_=ps)

                # ---------- phase B+D+E: LN(ctx) -> cnT -> kv chunks -----
                cnT = cnp.tile([128, KC, J], BF16)  # 24KB/part
                for n in range(JT // 4):
                    for j, ct in enumerate(ln_group(ctx_d, 4 * n, iop, stp)):
                        transpose_768(ct, cnT, 4 * n + j)
                    if True:
                        # kv for the 512 ctx rows whose cnT is now complete
                        sl = slice(n * 512, (n + 1) * 512)
                        ps = mmp.tile([128, 512], F32, tag="mm")
                        for k in range(KC):
                            nc.tensor.matmul(
                                ps,
                                lhsT=wkv_sb[:, k, :],
                                rhs=cnT[:, k, sl],
                                start=(k == 0),
                                stop=(k == KC - 1),
                            )
                        nc.vector.tensor_copy(out=kTd_lo[0:64, sl], in_=ps[0:64, :])
                        nc.scalar.copy(out=kTd_hi[64:128, sl], in_=ps[0:64, :])
                        nc.scalar.copy(out=vT[:, sl], in_=ps[64:128, :])
                        # v to token-major (vaug) as soon as each kv chunk lands
                        pt = tpp.tile([128, 4 * DH], BF16, tag="tp")
                        for jj in range(4):
                            nc.tensor.matmul(
                                pt[:, jj * DH : (jj + 1) * DH],
                                lhsT=vT[:, (4 * n + jj) * 128 : (4 * n + jj + 1) * 128],
                                rhs=ident[0:64, 0:64],
                                is_transpose=True,
                            )
                        nc.scalar.copy(
                            out=vaug[:, 4 * n : 4 * n + 4, 0:DH],
                            in_=pt.rearrange("p (j c) -> p j c", c=DH),
                        )

            # prefetch Wo during attention (pool opened before attn_data)
            wo_sb = wop.tile([128, KC, DIM], BF16)  # 9KB/part
            nc.sync.dma_start(out=wo_sb, in_=wo_r)

            # ---------- phase F+H: attention with FF interleaved ---------
            with (
                tc.tile_pool(name="spsum", bufs=4, space="PSUM") as spsum,
                tc.tile_pool(name="opsum", bufs=1, space="PSUM") as opsum,
                tc.tile_pool(name="hps", bufs=1, space="PSUM") as hps,
                tc.tile_pool(name="attn", bufs=5) as apool,
                tc.tile_pool(name="rb", bufs=2) as rbp,
                tc.tile_pool(name="wff1", bufs=4) as w1p,
                tc.tile_pool(name="sil", bufs=3) as silp,
            ):
                def ff_chunk(f):
                    # one 128-col chunk of the SwiGLU hidden; fills the PE
                    # while the scalar engine chews the softmax Exps
                    nc.sync.dma_start(out=w2_sb[:, f, :], in_=wff2_r[:, f, :])
                    wg = w1p.tile([128, KC, 128], BF16, tag="wg")
                    nc.sync.dma_start(
                        out=wg,
                        in_=wff1_r[:, :, FFI + f * 128 : FFI + (f + 1) * 128],
                    )
                    pg = hps.tile([128, 512], F32, tag="pg")
                    for k in range(KC):
                        nc.tensor.matmul(
                            pg,
                            lhsT=wg[:, k, :],
                            rhs=xnT[:, k, :],
                            start=(k == 0),
                            stop=(k == KC - 1),
                        )
                    sil = silp.tile([128, 512], BF16, tag="sil")
                    nc.scalar.activation(out=sil, in_=pg, func=AF.Silu)
                    wu = w1p.tile([128, KC, 128], BF16, tag="wu")
                    nc.sync.dma_start(
                        out=wu, in_=wff1_r[:, :, f * 128 : (f + 1) * 128]
                    )
                    pu = hps.tile([128, 512], F32, tag="pu")
                    for k in range(KC):
                        nc.tensor.matmul(
                            pu,
                            lhsT=wu[:, k, :],
                            rhs=xnT[:, k, :],
                            start=(k == 0),
                            stop=(k == KC - 1),
                        )
                    nc.vector.tensor_mul(out=hT[:, f, :], in0=pu, in1=sil)

                # Schraudolph fast-exp constants for bf16: bitcast of
                # int16(x*184.665 + 16249) approximates e^x to ~3% — the
                # random per-weight error washes out in the softmax average.
                S_A = 184.6650292
                S_B = 16249.0

                for hp in range(HEADS // 2):
                    po = [
                        opsum.tile([128, 512], F32, tag=f"o{i}", name=f"po{hp}_{i}")
                        for i in range(2)
                    ]
                    # blocks of 4 j-tiles: 8 consecutive sim matmuls, then
                    # per-head runs of 4 consecutive av matmuls into one
                    # PSUM bank (consecutive same-group matmuls run at the
                    # chain rate; per-instruction group/bank alternation
                    # measured ~100ns/MM slower)
                    JB = 4
                    for jb in range(0, JT, JB):
                        ats = {}
                        for jt in range(jb, jb + JB):
                            for i in range(2):
                                kTd = kTd_lo if i == 0 else kTd_hi
                                ps = spsum.tile([128, 512], F32, tag="s")
                                nc.tensor.matmul(
                                    ps,
                                    lhsT=kTd[:, jt * 128 : (jt + 1) * 128],
                                    rhs=qT[:, hp, :],
                                    start=True,
                                    stop=True,
                                )
                                if i == 0:
                                    # scalar-engine exact exp
                                    at = apool.tile([128, 512], BF16, tag="a0")
                                    nc.scalar.activation(
                                        out=at, in_=ps, func=AF.Exp
                                    )
                                else:
                                    # vector-engine fast exp (Schraudolph)
                                    ai = apool.tile([128, 512], I16, tag="a1")
                                    nc.vector.tensor_scalar(
                                        out=ai, in0=ps, scalar1=S_A,
                                        scalar2=S_B, op0=MULT, op1=ADD,
                                    )
                                    at = ai.bitcast(BF16)
                                ats[(jt, i)] = at
                        for i in range(2):
                            for jt in range(jb, jb + JB):
                                nc.tensor.matmul(
                                    po[i],
                                    lhsT=vaug[:, jt, :],
                                    rhs=ats[(jt, i)],
                                    start=(jt == 0),
                                    stop=(jt == JT - 1),
                                )
                    # 4 FF chunks first: ~13us of PE chains during which
                    # the scalar/vector queues drain, so the tail below
                    # frees the po banks well before hp+1's first AV
                    for f in range(4 * hp, 4 * hp + 4):
                        ff_chunk(f)
                    # tail: scalar copies pack both heads' attn-out and
                    # denom rows (freeing the po banks fast), then ONE
                    # broadcast matmul + reciprocal + multiply normalizes
                    # both heads at once
                    pos = rbp.tile([128, 512], F32, tag="pos")
                    sums2 = rbp.tile([128, 512], BF16, tag="sums2")
                    for i in range(2):
                        nc.scalar.copy(
                            out=pos[i * 64 : (i + 1) * 64, :], in_=po[i][0:DH, :]
                        )
                        nc.scalar.copy(
                            out=sums2[i * 64 : i * 64 + 1, :],
                            in_=po[i][DH : DH + 1, :],
                        )
                    pb = spsum.tile([128, 512], F32, tag="s")
                    nc.tensor.matmul(
                        pb,
                        lhsT=eye2,
                        rhs=sums2,
                        start=True,
                        stop=True,
                    )
                    rb = rbp.tile([128, 512], F32, tag="rbb")
                    nc.vector.reciprocal(out=rb, in_=pb)
                    nc.vector.tensor_mul(out=outT[:, hp, :], in0=pos, in1=rb)

        # ---------- phase G: out = outT.T@Wo + hT.T@Wff2 -----------------
        NCH = ((0, 512), (512, 256))  # 768 = 512 + 256, bank-aligned slices
        with (
            tc.tile_pool(name="gps", bufs=2, space="PSUM") as gps,
            tc.tile_pool(name="yout", bufs=2) as yp,
        ):
            for t in range(TT):
                pg = gps.tile([128, DIM], F32, tag="g")
                for n0, nw in NCH:
                    # Wff2 part first: hT is ready before the attention
                    # tail, so these chains don't wait on outT
                    for f in range(FH):
                        nc.tensor.matmul(
                            pg[:, n0 : n0 + nw],
                            lhsT=hT[:, f, t * 128 : (t + 1) * 128],
                            rhs=w2_sb[:, f, n0 : n0 + nw],
                            start=(f == 0),
                            stop=False,
                        )
                    for k in range(KC):
                        nc.tensor.matmul(
                            pg[:, n0 : n0 + nw],
                            lhsT=outT[:, k, t * 128 : (t + 1) * 128],
                            rhs=wo_sb[:, k, n0 : n0 + nw],
                            start=False,
                            stop=(k == KC - 1),
                        )
                ysb = yp.tile([128, DIM], F32, tag="y")
                nc.vector.tensor_copy(out=ysb, in_=pg)
                nc.sync.dma_start(out=y_d[t * 128 : (t + 1) * 128, :], in_=ysb)


@functools.lru_cache(maxsize=1)
def _build():
    _install_drain_patch()
    nc = bass.Bass("TRN2", target_bir_lowering=False, debug=False, num_devices=NCORES)
    return _emit(nc)


# ------------------------------------------------------ ntff hook shim
def _ensure_ntff_hook():
    """This image's `antenv` lacks `axon_hooks`; synthesize it so
    run_bass_kernel_spmd(trace=True) can capture NTFF profiles via the
    libaxon_pjrt C ABI (same recipe as trn_boot._ntff_profile_via_ctypes)."""
    import contextlib
    import ctypes
    import os
    import sys
    import types

    try:
        from antenv.axon_hooks import get_axon_ntff_profile_hook  # noqa: F401

        return
    except ImportError:
        pass
    import antenv

    mod = types.ModuleType("antenv.axon_hooks")
    holder = {"hook": None}
    mod.set_axon_ntff_profile_hook = lambda h: holder.__setitem__("hook", h)
    mod.get_axon_ntff_profile_hook = lambda: holder["hook"]
    sys.modules["antenv.axon_hooks"] = mod
    antenv.axon_hooks = mod

    so_path = "/opt/axon/libaxon_pjrt.so"
    if not os.path.exists(so_path):
        return
    lib = ctypes.CDLL(so_path)
    if not hasattr(lib, "axon_start_nrt_profile"):
        return
    lib.axon_start_nrt_profile.argtypes = [
        ctypes.POINTER(ctypes.c_int64),
        ctypes.c_size_t,
    ]
    lib.axon_start_nrt_profile.restype = ctypes.c_int64
    lib.axon_stop_nrt_profile.argtypes = [ctypes.c_char_p]
    lib.axon_stop_nrt_profile.restype = ctypes.c_int64

    @contextlib.contextmanager
    def _hook(output_dir, device_ids):
        import jax

        jax.devices()
        if device_ids:
            ids = (ctypes.c_int64 * len(device_ids))(*device_ids)
            rc = lib.axon_start_nrt_profile(ids, len(device_ids))
        else:
            rc = lib.axon_start_nrt_profile(None, 0)
        if rc != 0:
            raise RuntimeError(f"axon_start_nrt_profile rc={rc}")
        try:
            yield
        finally:
            n = lib.axon_stop_nrt_profile(str(output_dir).encode())
            print(f"ntff profile: {n} file(s) written to {output_dir}")

    mod.set_axon_ntff_profile_hook(_hook)


# ---------------------------------------------------------------- entry
TRACE = False  # test harnesses can flip this to capture an NTFF profile
LAST = None
BF = ml_dtypes.bfloat16
IDENT = np.eye(128, dtype=BF)


def kernel(**inputs):
    x = np.asarray(inputs["x"], dtype=np.float32)
    context = np.asarray(inputs["context"], dtype=np.float32)
    gx = np.asarray(inputs["gamma_x"], dtype=np.float32)
    gc = np.asarray(inputs["gamma_c"], dtype=np.float32)
    scale = DH**-0.5
    # fold LN gammas and the attention scale into the first-layer weights
    wq = ((gx[:, None] * np.asarray(inputs["Wq"])) * scale).astype(BF)
    wkv = (gc[:, None] * np.asarray(inputs["Wkv"])).astype(BF)
    wff1 = (gx[:, None] * np.asarray(inputs["Wff1"])).astype(BF)
    wo = np.asarray(inputs["Wo"], dtype=np.float32).astype(BF)
    wff2 = np.asarray(inputs["Wff2"], dtype=np.float32).astype(BF)
    xb = np.ascontiguousarray(x.astype(BF))
    cb = np.ascontiguousarray(context.astype(BF))

    in_maps = []
    for c in range(NCORES):
        b, t0 = c // 2, (c % 2) * TPC
        in_maps.append(
            {
                "x": np.ascontiguousarray(xb[b, t0 : t0 + TPC]),
                "ctx": cb[b],
                "wq": wq,
                "wkv": wkv,
                "wo": wo,
                "wff1": wff1,
                "wff2": wff2,
                "ident": IDENT,
            }
        )

    nc = _build()
    if TRACE:
        _ensure_ntff_hook()
    res = run_bass_kernel_spmd(nc, in_maps, list(range(NCORES)), trace=TRACE)
    global LAST
    LAST = res
    out = np.empty((B, N, DIM), np.float32)
    for c in range(NCORES):
        b, t0 = c // 2, (c % 2) * TPC
        out[b, t0 : t0 + TPC] = res.results[c]["y"]
    return out


# revision 22
# speedup vs baseline: 1.1996x; 1.0098x over previous
"""Trainium2 Bass kernel for nn_CrossAttention_8435315769719.

CrossAttention block: LN(x), LN(context), 12-head query / single shared
KV head cross-attention, output projection, plus a parallel SwiGLU FF on
the normed x.

Sharding: the 4096 query tokens (4 batches x 1024) are split into 8
row-slices of 512 tokens; core c handles batch c//2, token rows
(c%2)*512..+512, with that batch's full context. KV projection is
recomputed per core (cheap); no collectives needed.

All matmul operands are bf16 (same PE rate as float32r at N>=256 but
half the DMA/SBUF footprint); accumulation stays fp32 in PSUM. LN
statistics are computed in fp32. The SwiGLU FF chunks are interleaved
into the attention phase to keep the PE dense (and at max p-state)
while the scalar engine works through the softmax Exps.
"""

import functools

import numpy as np
import ml_dtypes

import concourse.bass as bass
import concourse.tile as tile
from concourse import mybir
from concourse.bass_utils import run_bass_kernel_spmd

# ---------------------------------------------------------------- sizes
DIM = 768
HEADS = 12
DH = 64
FFI = 3072  # FF inner (per u/gate half)
B = 4
N = 1024
J = 2048  # context length
EPS = 1e-5
NCORES = 8
TPC = 512  # query tokens per core

KC = DIM // 128  # 6 contraction chunks
TT = TPC // 128  # 4 token tiles per core
JT = J // 128  # 16 context tiles
FH = FFI // 128  # 24 ff tiles per half

F32 = mybir.dt.float32
F32R = mybir.dt.float32r
BF16 = mybir.dt.bfloat16
I16 = mybir.dt.int16

SUB = mybir.AluOpType.subtract
MULT = mybir.AluOpType.mult
ADD = mybir.AluOpType.add
AF = mybir.ActivationFunctionType


# ------------------------------------------------- tile drain workaround
def _install_drain_patch():
    """walrus in this container rejects >1 sync-wait on the Tile tail
    Drain ("Too many sync wait commands"). Split the final global-clock
    waits onto individual SP nops instead."""
    import concourse.tile as _t

    if getattr(_t.TileContext, "_drain_patched", False):
        return

    def _patched(self, tick_clock, wait_clock):
        nc = self.nc
        drain_inst = nc.sync.drain()
        wait_clock.add_sem_waits(
            drain_inst.ins, _t.ScopedClock({None: tick_clock.global_clock})
        )
        si = drain_inst.ins.sync_info
        waits = list(si.on_wait) if si is not None else []
        if si is not None and len(waits) > 1:
            si.on_wait = []
            for w in waits:
                n = nc.sync.nop(nofuse=True, hint="drain_split")
                n.ins.sync_info = mybir.SyncInfo(on_wait=[w], on_update=[])
        nc.all_engine_barrier()
        assert self.sems is not None
        popped = nc._tile_sem_poison_stack.pop()
        assert popped is self._sem_poison
        nc.clear_and_free_semaphores(list(self.sems.allocated().values()))
        nc.all_engine_barrier()

    _t.TileContext._drain_and_barrier = _patched
    _t.TileContext._drain_patched = True


# ------------------------------------------------------------ LN helper
def _ln_stats(nc, pool, xt, eps_t):
    """bn_stats/bn_aggr mean+var over the 768-wide free dim (3x256
    subgroups), returns (mu, var) [128,1] APs."""
    xg = xt.rearrange("p (s d) -> p s d", d=256)
    nsub = xg.shape[1]
    stats = pool.tile([128, nsub, nc.vector.BN_STATS_DIM], F32, tag="bnst")
    for s in range(nsub):
        nc.vector.bn_stats(out=stats[:, s, :], in_=xg[:, s, :])
    mv = pool.tile([128, nc.vector.BN_AGGR_DIM], F32, tag="mv")
    nc.vector.bn_aggr(out=mv, in_=stats)
    return mv[:, 0:1], mv[:, 1:2]


# -------------------------------------------------------- program build
def _emit(nc):
    x_d = nc.dram_tensor("x", [TPC, DIM], BF16, kind="ExternalInput").ap()
    ctx_d = nc.dram_tensor("ctx", [J, DIM], BF16, kind="ExternalInput").ap()
    wq_d = nc.dram_tensor("wq", [DIM, DIM], BF16, kind="ExternalInput").ap()
    wkv_d = nc.dram_tensor("wkv", [DIM, 2 * DH], BF16, kind="ExternalInput").ap()
    wo_d = nc.dram_tensor("wo", [DIM, DIM], BF16, kind="ExternalInput").ap()
    wff1_d = nc.dram_tensor("wff1", [DIM, 2 * FFI], BF16, kind="ExternalInput").ap()
    wff2_d = nc.dram_tensor("wff2", [FFI, DIM], BF16, kind="ExternalInput").ap()
    ident_d = nc.dram_tensor("ident", [128, 128], BF16, kind="ExternalInput").ap()
    y_d = nc.dram_tensor("y", [TPC, DIM], F32, kind="ExternalOutput").ap()

    wq_r = wq_d.rearrange("(k p) n -> p k n", p=128)
    wkv_r = wkv_d.rearrange("(k p) n -> p k n", p=128)
    wo_r = wo_d.rearrange("(k p) n -> p k n", p=128)
    wff1_r = wff1_d.rearrange("(k p) n -> p k n", p=128)

    with tile.TileContext(nc) as tc:
        _build_tile(tc, nc, x_d, ctx_d, wq_r, wkv_r, wo_r, wff1_r, wff2_d, y_d,
                    ident_d)
    _split_excess_waits(nc)
    return nc


def _split_excess_waits(nc, max_waits=1):
    """walrus in this container rejects instructions carrying more than
    ~2 sync waits ("Too many sync wait commands"). Move the excess onto
    same-engine NOPs inserted immediately before the instruction."""
    for fn in nc.m.functions:
        for bb in fn.blocks:
            rebuilt = []
            changed = False
            for inst in bb.instructions:
                si = inst.sync_info
                waits = list(si.on_wait) if si is not None else []
                if len(waits) > max_waits:
                    changed = True
                    si.on_wait = waits[-max_waits:]
                    excess = waits[:-max_waits]
                    for i0 in range(0, len(excess), max_waits):
                        nop = mybir.InstNoOp(
                            name=nc.get_next_instruction_name(),
                            sync_info=mybir.SyncInfo(
                                on_wait=excess[i0 : i0 + max_waits], on_update=[]
                            ),
                            bass_nofuse=True,
                            engine=inst.engine,
                        )
                        nc.register_instruction(nop, overwrite=True)
                        rebuilt.append(nop)
                rebuilt.append(inst)
            if changed:
                bb.instructions = rebuilt


def _build_tile(tc, nc, x_d, ctx_d, wq_r, wkv_r, wo_r, wff1_r, wff2_d, y_d,
                ident_d):
    from contextlib import ExitStack

    ctx = ExitStack()
    with ctx:
        constp = ctx.enter_context(tc.tile_pool(name="const", bufs=1))
        pers = ctx.enter_context(tc.tile_pool(name="pers", bufs=1))

        ident = constp.tile([128, 128], BF16)
        nc.sync.dma_start(out=ident, in_=ident_d)
        eps_t = constp.tile([128, 1], F32)
        nc.vector.memset(eps_t, EPS)
        # eye2 @ sums2 broadcasts head0's denom row to partitions 0:64
        # and head1's to 64:128, in a single matmul
        eye2 = constp.tile([128, 128], BF16)
        nc.gpsimd.memset(eye2, 0.0)
        nc.gpsimd.memset(eye2[0:1, 0:DH], 1.0)
        nc.gpsimd.memset(eye2[64:65, DH:128], 1.0)

        xnT = pers.tile([128, KC, TPC], BF16)  # LN(x)^T     6KB/part
        outT = pers.tile([128, KC, TPC], BF16)  # attn out^T  6KB/part

        def ln_group(dram, t0, iop, stp):
            # LN for 4 row-tiles with ONE batched reciprocal (InstReciprocal
            # has a flat ~1.3us cost regardless of free size)
            parts = []
            std4 = stp.tile([128, 4], F32, tag="std4")
            for j in range(4):
                t = t0 + j
                xt = iop.tile([128, DIM], BF16, tag="ln_in", name=f"ln{t}")
                nc.sync.dma_start(out=xt, in_=dram[t * 128 : (t + 1) * 128, :])
                mu, var = _ln_stats(nc, stp, xt, eps_t)
                nc.scalar.activation(
                    out=std4[:, j : j + 1], in_=var, func=AF.Sqrt,
                    bias=eps_t, scale=1.0,
                )
                parts.append((xt, mu))
            rs4 = stp.tile([128, 4], F32, tag="rs4")
            nc.vector.reciprocal(out=rs4, in_=std4)
            outs = []
            for j, (xt, mu) in enumerate(parts):
                nc.vector.tensor_scalar(
                    out=xt, in0=xt, scalar1=mu, scalar2=rs4[:, j : j + 1],
                    op0=SUB, op1=MULT,
                )
                outs.append(xt)
            return outs

        wop = ctx.enter_context(tc.tile_pool(name="wo", bufs=1))
        # SwiGLU hidden and Wff2 outlive the attention scope (used in G)
        htp = ctx.enter_context(tc.tile_pool(name="ht", bufs=1))
        hT = htp.tile([128, FH, TPC], BF16)  # swiglu hidden^T 24KB/part
        w2p = ctx.enter_context(tc.tile_pool(name="wff2", bufs=1))
        w2_sb = w2p.tile([128, FH, DIM], BF16)  # 36KB/part
        wff2_r = wff2_d.rearrange("(f p) n -> p f n", p=128)

        # attention working set, freed after phase F
        with tc.tile_pool(name="attn_data", bufs=1) as adp:
            qT = adp.tile([128, KC, TPC], BF16)  # q^T heads   6KB/part
            vT = adp.tile([64, J], BF16)  # v^T (dim-major)
            # k^T zero-padded to K=128: lo = [k; 0], hi = [0; k].
            # (K=64 / M=65 matmuls measured ~2.5x slower than K=M=128.)
            kTd_lo = adp.tile([128, J], BF16)
            kTd_hi = adp.tile([128, J], BF16)
            vaug = adp.tile([128, JT, 2 * DH], BF16)  # v | ones | zero pad
            # pad memsets on gpsimd: keeps the vector queue free for LN
            nc.gpsimd.memset(kTd_lo[64:128, :], 0.0)
            nc.gpsimd.memset(kTd_hi[0:64, :], 0.0)
            nc.gpsimd.memset(vaug, 0.0)
            nc.gpsimd.memset(vaug[:, :, DH : DH + 1], 1.0)

            with (
                tc.tile_pool(name="io", bufs=6) as iop,
                tc.tile_pool(name="stats", bufs=8) as stp,
                tc.tile_pool(name="tpp", bufs=4, space="PSUM") as tpp,
                tc.tile_pool(name="mmp", bufs=2, space="PSUM") as mmp,
                tc.tile_pool(name="wq", bufs=1) as wqp,
                tc.tile_pool(name="cnT", bufs=1) as cnp,
            ):
                def transpose_768(xt, dst, t):
                    # 6 transposes into one PSUM bank, one strided copy out
                    pt = tpp.tile([128, DIM], BF16, tag="tp")
                    for k in range(KC):
                        nc.tensor.matmul(
                            pt[:, k * 128 : (k + 1) * 128],
                            lhsT=xt[:, k * 128 : (k + 1) * 128],
                            rhs=ident,
                            is_transpose=True,
                        )
                    cp = nc.scalar.copy if t % 2 else nc.vector.tensor_copy
                    cp(
                        out=dst[:, :, t * 128 : (t + 1) * 128],
                        in_=pt.rearrange("p (k c) -> p k c", c=128),
                    )

                # ---------- phase A: LN(x) -> xnT ------------------------
                for t, xt in enumerate(ln_group(x_d, 0, iop, stp)):
                    transpose_768(xt, xnT, t)

                # weights: wq right after x tiles, before the ctx stream
                wq_sb = wqp.tile([128, KC, DIM], BF16)  # 9KB/part
                nc.sync.dma_start(out=wq_sb, in_=wq_r)
                wkv_sb = constp.tile([128, KC, 2 * DH], BF16)
                nc.sync.dma_start(out=wkv_sb, in_=wkv_r)

                # ---------- phase C: qT = Wq^T @ xn^T --------------------
                for i in range(KC):
                    ps = mmp.tile([128, 512], F32, tag="mm")
                    for k in range(KC):
                        nc.tensor.matmul(
                            ps,
                            lhsT=wq_sb[:, k, i * 128 : (i + 1) * 128],
                            rhs=xnT[:, k, :],
                            start=(k == 0),
                            stop=(k == KC - 1),
                        )
                    nc.vector.tensor_copy(out=qT[:, i, :], in_=ps)

                # ---------- phase B+D+E: LN(ctx) -> cnT -> kv chunks -----
                cnT = cnp.tile([128, KC, J], BF16)  # 24KB/part

                def v_to_vaug(n):
                    # v rows of kv chunk n to token-major vaug (deferred one
                    # chunk so the vT copy is long done when these run)
                    pt = tpp.tile([128, 4 * DH], BF16, tag="tp")
                    for jj in range(4):
                        nc.tensor.matmul(
                            pt[:, jj * DH : (jj + 1) * DH],
                            lhsT=vT[:, (4 * n + jj) * 128 : (4 * n + jj + 1) * 128],
                            rhs=ident[0:64, 0:64],
                            is_transpose=True,
                        )
                    nc.scalar.copy(
                        out=vaug[:, 4 * n : 4 * n + 4, 0:DH],
                        in_=pt.rearrange("p (j c) -> p j c", c=DH),
                    )

                for n in range(JT // 4):
                    for j, ct in enumerate(ln_group(ctx_d, 4 * n, iop, stp)):
                        transpose_768(ct, cnT, 4 * n + j)
                    # kv for the 512 ctx rows whose cnT is now complete
                    sl = slice(n * 512, (n + 1) * 512)
                    ps = mmp.tile([128, 512], F32, tag="mm")
                    for k in range(KC):
                        nc.tensor.matmul(
                            ps,
                            lhsT=wkv_sb[:, k, :],
                            rhs=cnT[:, k, sl],
                            start=(k == 0),
                            stop=(k == KC - 1),
                        )
                    nc.vector.tensor_copy(out=kTd_lo[0:64, sl], in_=ps[0:64, :])
                    nc.scalar.copy(out=kTd_hi[64:128, sl], in_=ps[0:64, :])
                    nc.vector.tensor_copy(out=vT[:, sl], in_=ps[64:128, :])
                    if n > 0:
                        v_to_vaug(n - 1)
                v_to_vaug(JT // 4 - 1)

            # prefetch Wo during attention (pool opened before attn_data)
            wo_sb = wop.tile([128, KC, DIM], BF16)  # 9KB/part
            nc.sync.dma_start(out=wo_sb, in_=wo_r)

            # ---------- phase F+H: attention with FF interleaved ---------
            with (
                tc.tile_pool(name="spsum", bufs=4, space="PSUM") as spsum,
                tc.tile_pool(name="opsum", bufs=1, space="PSUM") as opsum,
                tc.tile_pool(name="hps", bufs=1, space="PSUM") as hps,
                tc.tile_pool(name="attn", bufs=5) as apool,
                tc.tile_pool(name="rb", bufs=2) as rbp,
                tc.tile_pool(name="wff1", bufs=4) as w1p,
                tc.tile_pool(name="sil", bufs=3) as silp,
            ):
                def ff_chunk(f):
                    # one 128-col chunk of the SwiGLU hidden; fills the PE
                    # while the scalar engine chews the softmax Exps
                    nc.sync.dma_start(out=w2_sb[:, f, :], in_=wff2_r[:, f, :])
                    wg = w1p.tile([128, KC, 128], BF16, tag="wg")
                    nc.sync.dma_start(
                        out=wg,
                        in_=wff1_r[:, :, FFI + f * 128 : FFI + (f + 1) * 128],
                    )
                    pg = hps.tile([128, 512], F32, tag="pg")
                    for k in range(KC):
                        nc.tensor.matmul(
                            pg,
                            lhsT=wg[:, k, :],
                            rhs=xnT[:, k, :],
                            start=(k == 0),
                            stop=(k == KC - 1),
                        )
                    sil = silp.tile([128, 512], BF16, tag="sil")
                    nc.scalar.activation(out=sil, in_=pg, func=AF.Silu)
                    wu = w1p.tile([128, KC, 128], BF16, tag="wu")
                    nc.sync.dma_start(
                        out=wu, in_=wff1_r[:, :, f * 128 : (f + 1) * 128]
                    )
                    pu = hps.tile([128, 512], F32, tag="pu")
                    for k in range(KC):
                        nc.tensor.matmul(
                            pu,
                            lhsT=wu[:, k, :],
                            rhs=xnT[:, k, :],
                            start=(k == 0),
                            stop=(k == KC - 1),
                        )
                    nc.vector.tensor_mul(out=hT[:, f, :], in0=pu, in1=sil)

                # Schraudolph fast-exp constants for bf16: bitcast of
                # int16(x*184.665 + 16249) approximates e^x to ~3% — the
                # random per-weight error washes out in the softmax average.
                S_A = 184.6650292
                S_B = 16249.0

                for hp in range(HEADS // 2):
                    po = [
                        opsum.tile([128, 512], F32, tag=f"o{i}", name=f"po{hp}_{i}")
                        for i in range(2)
                    ]
                    # blocks of 4 j-tiles: 8 consecutive sim matmuls, then
                    # per-head runs of 4 consecutive av matmuls into one
                    # PSUM bank (consecutive same-group matmuls run at the
                    # chain rate; per-instruction group/bank alternation
                    # measured ~100ns/MM slower)
                    JB = 4
                    for jb in range(0, JT, JB):
                        ats = {}
                        for jt in range(jb, jb + JB):
                            for i in range(2):
                                kTd = kTd_lo if i == 0 else kTd_hi
                                ps = spsum.tile([128, 512], F32, tag="s")
                                nc.tensor.matmul(
                                    ps,
                                    lhsT=kTd[:, jt * 128 : (jt + 1) * 128],
                                    rhs=qT[:, hp, :],
                                    start=True,
                                    stop=True,
                                )
                                if i == 0:
                                    # scalar-engine exact exp
                                    at = apool.tile([128, 512], BF16, tag="a0")
                                    nc.scalar.activation(
                                        out=at, in_=ps, func=AF.Exp
                                    )
                                else:
                                    # vector-engine fast exp (Schraudolph)
                                    ai = apool.tile([128, 512], I16, tag="a1")
                                    nc.vector.tensor_scalar(
                                        out=ai, in0=ps, scalar1=S_A,
                                        scalar2=S_B, op0=MULT, op1=ADD,
                                    )
                                    at = ai.bitcast(BF16)
                                ats[(jt, i)] = at
                        for i in range(2):
                            for jt in range(jb, jb + JB):
                                nc.tensor.matmul(
                                    po[i],
                                    lhsT=vaug[:, jt, :],
                                    rhs=ats[(jt, i)],
                                    start=(jt == 0),
                                    stop=(jt == JT - 1),
                                )
                    # 4 FF chunks first: ~13us of PE chains during which
                    # the scalar/vector queues drain, so the tail below
                    # frees the po banks well before hp+1's first AV
                    for f in range(4 * hp, 4 * hp + 4):
                        ff_chunk(f)
                    # tail: scalar copies pack both heads' attn-out and
                    # denom rows (freeing the po banks fast), then ONE
                    # broadcast matmul + reciprocal + multiply normalizes
                    # both heads at once
                    pos = rbp.tile([128, 512], F32, tag="pos")
                    sums2 = rbp.tile([128, 512], BF16, tag="sums2")
                    for i in range(2):
                        nc.scalar.copy(
                            out=pos[i * 64 : (i + 1) * 64, :], in_=po[i][0:DH, :]
                        )
                        nc.scalar.copy(
                            out=sums2[i * 64 : i * 64 + 1, :],
                            in_=po[i][DH : DH + 1, :],
                        )
                    pb = hps.tile([128, 512], F32, tag="pg")
                    nc.tensor.matmul(
                        pb,
                        lhsT=eye2,
                        rhs=sums2,
                        start=True,
                        stop=True,
                    )
                    rb = rbp.tile([128, 512], F32, tag="rbb")
                    nc.vector.reciprocal(out=rb, in_=pb)
                    nc.vector.tensor_mul(out=outT[:, hp, :], in0=pos, in1=rb)

        # ---------- phase G: out = outT.T@Wo + hT.T@Wff2 -----------------
        NCH = ((0, 512), (512, 256))  # 768 = 512 + 256, bank-aligned slices
        with (
            tc.tile_pool(name="gps", bufs=2, space="PSUM") as gps,
            tc.tile_pool(name="yout", bufs=2) as yp,
        ):
            for t in range(TT):
                pg = gps.tile([128, DIM], F32, tag="g")
                for n0, nw in NCH:
                    # Wff2 part first: hT is ready before the attention
                    # tail, so these chains don't wait on outT
                    for f in range(FH):
                        nc.tensor.matmul(
                            pg[:, n0 : n0 + nw],
                            lhsT=hT[:, f, t * 128 : (t + 1) * 128],
                            rhs=w2_sb[:, f, n0 : n0 + nw],
                            start=(f == 0),
                            stop=False,
                        )
                    for k in range(KC):
                        nc.tensor.matmul(
                            pg[:, n0 : n0 + nw],
                            lhsT=outT[:, k, t * 128 : (t + 1) * 128],
                            rhs=wo_sb[:, k, n0 : n0 + nw],
                            start=False,
                            stop=(k == KC - 1),
                        )
                ysb = yp.tile([128, DIM], F32, tag="y")
                nc.vector.tensor_copy(out=ysb, in_=pg)
                nc.sync.dma_start(out=y_d[t * 128 : (t + 1) * 128, :], in_=ysb)


@functools.lru_cache(maxsize=1)
def _build():
    _install_drain_patch()
    nc = bass.Bass("TRN2", target_bir_lowering=False, debug=False, num_devices=NCORES)
    return _emit(nc)


# ------------------------------------------------------ ntff hook shim
def _ensure_ntff_hook():
    """This image's `antenv` lacks `axon_hooks`; synthesize it so
    run_bass_kernel_spmd(trace=True) can capture NTFF profiles via the
    libaxon_pjrt C ABI (same recipe as trn_boot._ntff_profile_via_ctypes)."""
    import contextlib
    import ctypes
    import os
    import sys
    import types

    try:
        from antenv.axon_hooks import get_axon_ntff_profile_hook  # noqa: F401

        return
    except ImportError:
        pass
    import antenv

    mod = types.ModuleType("antenv.axon_hooks")
    holder = {"hook": None}
    mod.set_axon_ntff_profile_hook = lambda h: holder.__setitem__("hook", h)
    mod.get_axon_ntff_profile_hook = lambda: holder["hook"]
    sys.modules["antenv.axon_hooks"] = mod
    antenv.axon_hooks = mod

    so_path = "/opt/axon/libaxon_pjrt.so"
    if not os.path.exists(so_path):
        return
    lib = ctypes.CDLL(so_path)
    if not hasattr(lib, "axon_start_nrt_profile"):
        return
    lib.axon_start_nrt_profile.argtypes = [
        ctypes.POINTER(ctypes.c_int64),
        ctypes.c_size_t,
    ]
    lib.axon_start_nrt_profile.restype = ctypes.c_int64
    lib.axon_stop_nrt_profile.argtypes = [ctypes.c_char_p]
    lib.axon_stop_nrt_profile.restype = ctypes.c_int64

    @contextlib.contextmanager
    def _hook(output_dir, device_ids):
        import jax

        jax.devices()
        if device_ids:
            ids = (ctypes.c_int64 * len(device_ids))(*device_ids)
            rc = lib.axon_start_nrt_profile(ids, len(device_ids))
        else:
            rc = lib.axon_start_nrt_profile(None, 0)
        if rc != 0:
            raise RuntimeError(f"axon_start_nrt_profile rc={rc}")
        try:
            yield
        finally:
            n = lib.axon_stop_nrt_profile(str(output_dir).encode())
            print(f"ntff profile: {n} file(s) written to {output_dir}")

    mod.set_axon_ntff_profile_hook(_hook)


# ---------------------------------------------------------------- entry
TRACE = False  # test harnesses can flip this to capture an NTFF profile
LAST = None
BF = ml_dtypes.bfloat16
IDENT = np.eye(128, dtype=BF)


def kernel(**inputs):
    x = np.asarray(inputs["x"], dtype=np.float32)
    context = np.asarray(inputs["context"], dtype=np.float32)
    gx = np.asarray(inputs["gamma_x"], dtype=np.float32)
    gc = np.asarray(inputs["gamma_c"], dtype=np.float32)
    scale = DH**-0.5
    # fold LN gammas and the attention scale into the first-layer weights
    wq = ((gx[:, None] * np.asarray(inputs["Wq"])) * scale).astype(BF)
    wkv = (gc[:, None] * np.asarray(inputs["Wkv"])).astype(BF)
    wff1 = (gx[:, None] * np.asarray(inputs["Wff1"])).astype(BF)
    wo = np.asarray(inputs["Wo"], dtype=np.float32).astype(BF)
    wff2 = np.asarray(inputs["Wff2"], dtype=np.float32).astype(BF)
    xb = np.ascontiguousarray(x.astype(BF))
    cb = np.ascontiguousarray(context.astype(BF))

    in_maps = []
    for c in range(NCORES):
        b, t0 = c // 2, (c % 2) * TPC
        in_maps.append(
            {
                "x": np.ascontiguousarray(xb[b, t0 : t0 + TPC]),
                "ctx": cb[b],
                "wq": wq,
                "wkv": wkv,
                "wo": wo,
                "wff1": wff1,
                "wff2": wff2,
                "ident": IDENT,
            }
        )

    nc = _build()
    if TRACE:
        _ensure_ntff_hook()
    res = run_bass_kernel_spmd(nc, in_maps, list(range(NCORES)), trace=TRACE)
    global LAST
    LAST = res
    out = np.empty((B, N, DIM), np.float32)
    for c in range(NCORES):
        b, t0 = c // 2, (c % 2) * TPC
        out[b, t0 : t0 + TPC] = res.results[c]["y"]
    return out
